# revision 14
# baseline (speedup 1.0000x reference)
"""Trainium2 Bass kernel for nn_MinamoTopoModel (3-layer GAT + mean-pool + FC).

Strategy (8 NeuronCores, SPMD):
  - Nodes partitioned contiguously across cores (2500/core); edges assigned by
    destination core so segment-softmax + scatter-add stay local.
  - Per layer: row-sharded dense matmul h = x @ W on each core, then AllGather
    of the "augmented" node rows [h | e_src] (bf16) so every core can gather
    arbitrary source rows, then dst-local attention:
      per 125-dst block (edges bin-packed + node ids permuted so every block
      is exactly T=6 tiles of 128 edges):
        phase 1: indirect-DMA gather of source rows, build one-hot (edge x
                 dst-slot) via iota compare, expand per-dst e_dst to edges with
                 a transposed-one-hot matmul, w = exp(leaky(es+ed)) (shift-
                 invariant softmax, max-subtraction skipped -- validated),
                 den += onehot^T @ w (PSUM accum).
        phase 2: alpha = w * (1/den)[dst], weighted features via broadcast
                 multiply, out += onehot^T @ (alpha * h_src) (PSUM accum).
      epilogue: + bias, leaky_relu -> x_next rows (bf16, row-major).
  - Next layer's lhsT tiles come from x_next via HWDGE DMA-transpose loads.
  - Layer 3 (heads=1) feeds a per-block matmul with graph one-hots ->
    per-core pooled sums [16, 512]; AllReduce; mean, FC, leaky -> out.

All floats bf16 except attention-scalar PSUM accumulation (fp32) and the final
FC epilogue; validated vs the jax reference at ~4e-3 max-rel error.
"""

import math
import os
from contextlib import ExitStack

import numpy as np
import ml_dtypes

import concourse.bass as bass
import concourse.bacc as bacc
import concourse.mybir as mybir
import concourse.tile as tile
from concourse.bass import IndirectOffsetOnAxis
from concourse.masks import make_identity

BF16 = mybir.dt.bfloat16
F32 = mybir.dt.float32
I32 = mybir.dt.int32
AX = mybir.AxisListType
OP = mybir.AluOpType
ACT_EXP = mybir.ActivationFunctionType.Exp

SLOPE = 0.2
G = 16  # graphs
NCORES = 8

bf16np = ml_dtypes.bfloat16


# --------------------------------------------------------------------------
# Host-side preprocessing
# --------------------------------------------------------------------------

def _pack_blocks(deg, nb, dpb, cap):
    """LPT bin-pack local dsts (by degree) into nb bins, each getting exactly
    <= dpb dsts and <= cap edges.  Returns per-bin dst lists or None."""
    order = np.argsort(-deg, kind="stable")
    loads = np.zeros(nb, dtype=np.int64)
    cnts = np.zeros(nb, dtype=np.int64)
    bins = [[] for _ in range(nb)]
    for d in order:
        g = deg[d]
        placed = False
        for i in np.argsort(loads, kind="stable"):
            if cnts[i] < dpb and loads[i] + g <= cap:
                loads[i] += g
                cnts[i] += 1
                bins[i].append(d)
                placed = True
                break
        if not placed:
            return None
    return bins


def preprocess(x, edge_index, batch, cfg):
    """Build per-core metadata + permutations. Returns (percore, hostinfo)."""
    N, NLOC, DPB, NB = cfg["N"], cfg["NLOC"], cfg["DPB"], cfg["NB"]
    ei = np.asarray(edge_index).astype(np.int64)
    bat = np.asarray(batch).astype(np.int64)
    x = np.asarray(x, dtype=np.float32)
    loop = np.arange(N, dtype=np.int64)
    src = np.concatenate([ei[0], loop])
    dst = np.concatenate([ei[1], loop])

    deg = np.bincount(dst, minlength=N)

    # choose T: tiles per block (uniform across all cores/blocks)
    T = max(1, math.ceil(deg.reshape(NCORES, NLOC).sum(1).max() / (NB * 128)))
    all_bins = None
    while True:
        cap = T * 128
        all_bins = []
        ok = True
        for c in range(NCORES):
            bins = _pack_blocks(deg[c * NLOC:(c + 1) * NLOC], NB, DPB, cap)
            if bins is None:
                ok = False
                break
            all_bins.append(bins)
        if ok:
            break
        T += 1
        assert T <= 64, "block packing failed"
    cfg = dict(cfg)
    cfg["T"] = T
    EPAD = NB * T * 128

    # permutation: per core, orig local id -> new local id
    perm = np.full((NCORES, NLOC), -1, dtype=np.int64)
    for c in range(NCORES):
        for b, bl in enumerate(all_bins[c]):
            for j, d in enumerate(bl):
                perm[c, d] = b * DPB + j
    assert (perm >= 0).all()
    # global row id of each original node in the AllGathered aug matrix
    owner = np.arange(N) // NLOC
    g2r = owner * NLOC + perm[owner, np.arange(N) % NLOC]

    # edges per core, grouped by (block, sorted by src row for HBM locality)
    ecore = dst // NLOC
    metas = []
    gone = []
    xT = []
    NROWS = cfg["NROWS"]
    for c in range(NCORES):
        m = np.zeros((EPAD, 2), dtype=np.int32)
        m[:, 1] = 999  # pad slot: matches no dst lane
        sel = np.nonzero(ecore == c)[0]
        nl = perm[c, dst[sel] - c * NLOC]       # new local dst id
        blk = nl // DPB
        slot = nl % DPB
        rows = g2r[src[sel]]
        order = np.lexsort((rows, blk))
        blk, slot, rows = blk[order], slot[order], rows[order]
        pos = 0
        for b in range(NB):
            cnt = int((blk == b).sum())
            base = b * T * 128
            m[base:base + cnt, 0] = rows[pos:pos + cnt]
            m[base:base + cnt, 1] = slot[pos:pos + cnt]
            pos += cnt
        metas.append(m)

        inv = np.empty(NLOC, dtype=np.int64)
        inv[perm[c]] = np.arange(NLOC)          # new local -> orig local
        orig = inv + c * NLOC
        go = np.zeros((NLOC, G), dtype=bf16np)
        go[np.arange(NLOC), bat[orig]] = 1.0
        gone.append(go)

        xt = np.zeros((x.shape[1], NROWS), dtype=np.float32)
        xt[:, :NLOC] = x[orig].T
        xT.append(xt)

    cnts = np.bincount(bat, minlength=G).astype(np.float32)
    crecip = (1.0 / np.maximum(cnts, 1.0)).reshape(G, 1).astype(np.float32)

    cfg["_perm"] = perm
    return cfg, metas, gone, xT, crecip


# --------------------------------------------------------------------------
# Bass program
# --------------------------------------------------------------------------

def build_program(cfg):
    N, NLOC, DPB, NB, T = cfg["N"], cfg["NLOC"], cfg["DPB"], cfg["NB"], cfg["T"]
    NROWS = cfg["NROWS"]
    MT = NROWS // 128
    EPAD = NB * T * 128
    TILEF, EMB, HID, OUT, FEAT, HEADS = 32, 128, 256, 512, 512, 8
    D1, D2, D3 = HEADS * HID, HEADS * HID, OUT   # 2048, 2048, 512
    RL12, RL3 = D1 + 16, D3 + 16                 # padded aug row lengths

    nc = bacc.Bacc(num_devices=NCORES)

    # ---------------- I/O ----------------
    xT_in = nc.dram_tensor("xT_in", [TILEF, NROWS], F32, kind="ExternalInput")
    meta_in = nc.dram_tensor("meta", [EPAD, 2], I32, kind="ExternalInput")
    gone_in = nc.dram_tensor("gone", [NLOC, G], BF16, kind="ExternalInput")
    crecip_in = nc.dram_tensor("crecip", [G, 1], F32, kind="ExternalInput")
    W0_in = nc.dram_tensor("W0", [TILEF, EMB], F32, kind="ExternalInput")
    b0_in = nc.dram_tensor("b0", [1, EMB], BF16, kind="ExternalInput")
    Ws_in = {}
    for nm, kdim, ndim in (("W1", EMB, D1), ("W2", D1, D2), ("W3", D2, D3),
                           ("Wf", OUT, FEAT)):
        Ws_in[nm] = nc.dram_tensor(nm, [kdim, ndim], BF16, kind="ExternalInput")
    asd_in = {1: nc.dram_tensor("asd1", [2, D1], BF16, kind="ExternalInput"),
              2: nc.dram_tensor("asd2", [2, D2], BF16, kind="ExternalInput"),
              3: nc.dram_tensor("asd3", [2, D3], BF16, kind="ExternalInput")}
    b_in = {1: nc.dram_tensor("b1", [1, D1], BF16, kind="ExternalInput"),
            2: nc.dram_tensor("b2", [1, D2], BF16, kind="ExternalInput"),
            3: nc.dram_tensor("b3", [1, D3], BF16, kind="ExternalInput")}
    bf_in = nc.dram_tensor("bfc", [1, FEAT], F32, kind="ExternalInput")
    out_ext = nc.dram_tensor("out", [G, FEAT], F32, kind="ExternalOutput")

    with tile.TileContext(nc) as tc, ExitStack() as ctx:
        dram = ctx.enter_context(tc.tile_pool(name="dram", bufs=1, space="DRAM"))
        cpool = ctx.enter_context(tc.tile_pool(name="consts", bufs=1))
        wpool = ctx.enter_context(tc.tile_pool(name="weights", bufs=1))
        sb = ctx.enter_context(tc.tile_pool(name="work", bufs=2))
        augp = ctx.enter_context(tc.tile_pool(name="augp", bufs=T + 1))
        ohp = ctx.enter_context(tc.tile_pool(name="ohp", bufs=T + 1))
        smp = ctx.enter_context(tc.tile_pool(name="smp", bufs=T + 2))
        pp_dense = ctx.enter_context(
            tc.tile_pool(name="pp_dense", bufs=1, space="PSUM"))
        pp_attn = ctx.enter_context(
            tc.tile_pool(name="pp_attn", bufs=1, space="PSUM"))
        pp_den = ctx.enter_context(
            tc.tile_pool(name="pp_den", bufs=1, space="PSUM"))
        pp_t = ctx.enter_context(tc.tile_pool(name="pp_t", bufs=2, space="PSUM"))

        # ------------- internal DRAM -------------
        x0 = dram.tile([NROWS, EMB], BF16, name="x0")
        x1 = dram.tile([NROWS, D1], BF16, name="x1")
        x2 = dram.tile([NROWS, D2], BF16, name="x2")
        xs = {0: x0, 1: x1, 2: x2}
        aug_l = {1: dram.tile([NLOC, RL12], BF16, name="aug1l"),
                 2: dram.tile([NLOC, RL12], BF16, name="aug2l"),
                 3: dram.tile([NLOC, RL3], BF16, name="aug3l")}
        aug_f = {1: dram.tile([N, RL12], BF16, name="aug1f"),
                 2: dram.tile([N, RL12], BF16, name="aug2f"),
                 3: dram.tile([N, RL3], BF16, name="aug3f")}
        pool_in = dram.tile([G, FEAT], F32, name="pool_in")
        pool_out = dram.tile([G, FEAT], F32, name="pool_out")

        # ------------- constants -------------
        ident = cpool.tile([128, 128], BF16, name="ident")
        make_identity(nc, ident[:])
        iota_i = cpool.tile([128, 128], I32, name="iota_i")
        nc.gpsimd.iota(iota_i[:], pattern=[[1, 128]], base=0, channel_multiplier=0)
        iota_f = cpool.tile([128, 128], F32, name="iota_f")
        nc.vector.tensor_copy(iota_f[:], iota_i[:])

        W0_sb = wpool.tile([TILEF, EMB], F32, name="W0sb")
        nc.sync.dma_start(W0_sb[:], W0_in[:, :])
        Wsb = {}
        for nm, kdim, ndim in (("W1", EMB, D1), ("W2", D1, D2), ("W3", D2, D3),
                               ("Wf", OUT, FEAT)):
            tiles = []
            for k in range(kdim // 128):
                t = wpool.tile([128, ndim], BF16, name=f"{nm}k{k}")
                nc.sync.dma_start(t[:], Ws_in[nm][k * 128:(k + 1) * 128, :])
                tiles.append(t)
            Wsb[nm] = tiles
        asd_sb, b_sb = {}, {}
        for li, dd in ((1, D1), (2, D2), (3, D3)):
            t = wpool.tile([128, 2, dd], BF16, name=f"asd{li}sb")
            nc.sync.dma_start(t[:], asd_in[li][None, :, :].to_broadcast([128, 2, dd]))
            asd_sb[li] = t
            bt = wpool.tile([128, dd], BF16, name=f"b{li}sb")
            nc.sync.dma_start(bt[:], b_in[li][:, :].to_broadcast([128, dd]))
            b_sb[li] = bt
        b0_sb = wpool.tile([128, EMB], BF16, name="b0sb")
        nc.sync.dma_start(b0_sb[:], b0_in[:, :].to_broadcast([128, EMB]))
        bf_sb = wpool.tile([G, FEAT], F32, name="bfsb")
        nc.sync.dma_start(bf_sb[:], bf_in[:, :].to_broadcast([G, FEAT]))
        crecip_sb = wpool.tile([G, 1], F32, name="crecipsb")
        nc.sync.dma_start(crecip_sb[:], crecip_in[:, :])
        gone_sb = wpool.tile([DPB, NB, G], BF16, name="gonesb")
        nc.sync.dma_start(
            gone_sb[:], gone_in[:, :].rearrange("(b p) g -> p b g", p=DPB))

        # x1/x2 tail rows (NLOC:NROWS) stay uninitialized: every consumer is
        # row-independent and tail rows are never stored to aug/output.

        def leaky(out_ap, in_ap, tmp_tile):
            nc.vector.tensor_scalar_mul(tmp_tile, in_ap, SLOPE)
            nc.vector.tensor_tensor(out=out_ap, in0=in_ap, in1=tmp_tile,
                                    op=OP.max)

        # =========== layer 0: x0 = leaky(x @ W0 + b0) ===========
        for m in range(MT):
            ms = slice(m * 128, (m + 1) * 128)
            lhs0 = sb.tile([TILEF, 128], F32, name="lhs0", tag="lhs0", bufs=2)
            nc.sync.dma_start(lhs0[:], xT_in[:, ms])
            ps = pp_dense.tile([128, 512], F32, name="ps0", tag="dps")
            nc.tensor.matmul(ps[:, :EMB], lhsT=lhs0[:], rhs=W0_sb[:],
                             start=True, stop=True)
            hraw = sb.tile([128, EMB], BF16, name="hraw0", tag="x0t", bufs=2)
            nc.vector.tensor_tensor(out=hraw[:], in0=ps[:, :EMB],
                                    in1=b0_sb[:], op=OP.add)
            tmp = sb.tile([128, EMB], BF16, name="tmp0", tag="x0tmp", bufs=2)
            xo = sb.tile([128, EMB], BF16, name="xo0", tag="x0o", bufs=2)
            leaky(xo[:], hraw[:], tmp[:])
            nc.sync.dma_start(x0[ms, :], xo[:])

        # =========== GAT layers ===========
        def gat_layer(li, d_in, dd, heads, rl, x_prev, x_out):
            ch = dd // heads
            W = Wsb[f"W{li}"]
            KT = d_in // 128
            NCH = dd // 512

            # ---- dense stage: h = x_prev @ W ; aug rows ; ed ----
            for m in range(MT):
                rows = min(128, NLOC - m * 128)
                if rows <= 0:
                    continue
                ms = slice(m * 128, (m + 1) * 128)
                lhsT = []
                for k in range(KT):
                    lt = sb.tile([128, 128], BF16, name=f"lhsT{li}",
                                 tag="lhsT", bufs=20)
                    nc.sync.dma_start(
                        lt[:], x_prev[ms, k * 128:(k + 1) * 128], transpose=True)
                    lhsT.append(lt)
                h_sb = sb.tile([128, dd], BF16, name=f"h{li}", tag="h_sb", bufs=2)
                for nchunk in range(NCH):
                    nsl = slice(nchunk * 512, (nchunk + 1) * 512)
                    ps = pp_dense.tile([128, 512], F32, name=f"dps{li}", tag="dps")
                    for k in range(KT):
                        nc.tensor.matmul(ps[:], lhsT=lhsT[k][:],
                                         rhs=W[k][:, nsl],
                                         start=(k == 0), stop=(k == KT - 1))
                    nc.vector.tensor_copy(h_sb[:, nsl], ps[:])
                # es/ed
                esd = sb.tile([128, 2, heads], F32, name=f"esd{li}",
                              tag="esd", bufs=3)
                mt = sb.tile([128, dd], BF16, name=f"esdm{li}", tag="sc2k", bufs=2)
                for j in range(2):
                    nc.vector.tensor_tensor(out=mt[:], in0=h_sb[:],
                                            in1=asd_sb[li][:, j, :], op=OP.mult)
                    nc.vector.tensor_reduce(
                        out=esd[:, j, :],
                        in_=mt[:].rearrange("p (h c) -> p h c", h=heads),
                        axis=AX.X, op=OP.add)
                esed = sb.tile([128, 16], BF16, name=f"esed{li}",
                               tag="esed", bufs=3)
                nc.vector.memset(esed[:], 0.0)
                nc.vector.tensor_copy(esed[:, 0:2 * heads],
                                      esd[:].rearrange("p a h -> p (a h)"))
                nc.sync.dma_start(aug_l[li][ms.start:ms.start + rows, 0:dd],
                                  h_sb[:rows, :])
                nc.sync.dma_start(aug_l[li][ms.start:ms.start + rows, dd:dd + 16],
                                  esed[:rows, :])

            # ---- AllGather aug rows ----
            nc.gpsimd.collective_compute(
                "AllGather", OP.bypass,
                replica_groups=[list(range(NCORES))],
                ins=[aug_l[li][:, :].opt()],
                outs=[aug_f[li][:, :].opt()])

            # ---- attention stage ----
            pool_ps = None
            if li == 3:
                pool_ps = pp_dense.tile([G, 512], F32, name="poolps", tag="dps")
            for b in range(NB):
                bs = slice(b * DPB, (b + 1) * DPB)
                ed_blk = sb.tile([DPB, heads], BF16, name=f"edblk{li}",
                                 tag="edblk", bufs=3)
                nc.sync.dma_start(ed_blk[:],
                                  aug_l[li][bs, dd + heads:dd + 2 * heads])
                den_ps = pp_den.tile([DPB, heads], F32, name=f"den{li}", tag="den")
                out_ps = pp_attn.tile([DPB, dd], F32, name=f"oat{li}", tag="oat")
                augs, ohs, ohTs, ws = [], [], [], []
                for t in range(T):
                    ti = b * T + t
                    es_ = slice(ti * 128, (ti + 1) * 128)
                    meta_t = smp.tile([128, 2], I32, name=f"meta{li}", tag="meta")
                    nc.sync.dma_start(meta_t[:], meta_in[es_, :])
                    aug_t = augp.tile([128, rl], BF16, name=f"aug{li}", tag="aug")
                    nc.gpsimd.indirect_dma_start(
                        out=aug_t[:], out_offset=None,
                        in_=aug_f[li][:, :],
                        in_offset=IndirectOffsetOnAxis(ap=meta_t[:, 0:1], axis=0))
                    slot_f = smp.tile([128, 1], F32, name=f"slot{li}", tag="slot")
                    nc.vector.tensor_copy(slot_f[:], meta_t[:, 1:2])
                    oh = ohp.tile([128, 128], BF16, name=f"oh{li}", tag="oh")
                    nc.vector.tensor_tensor(
                        out=oh[:], in0=slot_f[:].to_broadcast([128, 128]),
                        in1=iota_f[:], op=OP.is_equal)
                    ohT_ps = pp_t.tile([128, 128], BF16, name=f"ohTp{li}",
                                       tag="tps")
                    nc.tensor.transpose(ohT_ps[:], oh[:], ident[:])
                    ohT = ohp.tile([128, 128], BF16, name=f"ohT{li}", tag="ohT")
                    nc.vector.tensor_copy(ohT[:], ohT_ps[:])
                    edpe_ps = pp_t.tile([128, heads], F32, name=f"edpe{li}",
                                        tag="tps")
                    nc.tensor.matmul(edpe_ps[:], lhsT=ohT[:DPB, :],
                                     rhs=ed_blk[:], start=True, stop=True)
                    e_t = smp.tile([128, heads], F32, name=f"et{li}", tag="et")
                    nc.vector.tensor_tensor(out=e_t[:],
                                            in0=aug_t[:, dd:dd + heads],
                                            in1=edpe_ps[:], op=OP.add)
                    w1 = smp.tile([128, heads], F32, name=f"w1{li}", tag="w1")
                    w2 = smp.tile([128, heads], F32, name=f"w2{li}", tag="w2")
                    nc.scalar.activation(w1[:], e_t[:], ACT_EXP)
                    nc.scalar.activation(w2[:], e_t[:], ACT_EXP, scale=SLOPE)
                    w_t = smp.tile([128, heads], BF16, name=f"wt{li}", tag="wt")
                    nc.vector.tensor_tensor(out=w_t[:], in0=w1[:], in1=w2[:],
                                            op=OP.max)
                    nc.tensor.matmul(den_ps[:], lhsT=oh[:, :DPB], rhs=w_t[:],
                                     start=(t == 0), stop=(t == T - 1))
                    augs.append(aug_t)
                    ohs.append(oh)
                    ohTs.append(ohT)
                    ws.append(w_t)
                rd = sb.tile([DPB, heads], F32, name=f"rd{li}", tag="rd", bufs=2)
                nc.vector.reciprocal(rd[:], den_ps[:])
                rd_bf = sb.tile([DPB, heads], BF16, name=f"rdbf{li}",
                                tag="rdbf", bufs=2)
                nc.vector.tensor_copy(rd_bf[:], rd[:])
                for t in range(T):
                    rdpe_ps = pp_t.tile([128, heads], F32, name=f"rdpe{li}",
                                        tag="tps")
                    nc.tensor.matmul(rdpe_ps[:], lhsT=ohTs[t][:DPB, :],
                                     rhs=rd_bf[:], start=True, stop=True)
                    wn = smp.tile([128, heads], BF16, name=f"wn{li}", tag="wn")
                    nc.vector.tensor_tensor(out=wn[:], in0=ws[t][:],
                                            in1=rdpe_ps[:], op=OP.mult)
                    fw = sb.tile([128, dd], BF16, name=f"fw{li}", tag="sc2k", bufs=2)
                    nc.vector.tensor_tensor(
                        out=fw[:].rearrange("p (h c) -> p h c", h=heads),
                        in0=augs[t][:, 0:dd].rearrange("p (h c) -> p h c",
                                                       h=heads),
                        in1=wn[:].unsqueeze(2).to_broadcast([128, heads, ch]),
                        op=OP.mult)
                    for nchunk in range(NCH):
                        nsl = slice(nchunk * 512, (nchunk + 1) * 512)
                        nc.tensor.matmul(out_ps[:, nsl], lhsT=ohs[t][:, :DPB],
                                         rhs=fw[:, nsl],
                                         start=(t == 0), stop=(t == T - 1))
                # epilogue: + bias, leaky -> x_out rows (or pooling for L3)
                xr = sb.tile([DPB, dd], BF16, name=f"xr{li}", tag="xr", bufs=1)
                nc.vector.tensor_tensor(out=xr[:], in0=out_ps[:],
                                        in1=b_sb[li][:DPB, :], op=OP.add)
                xt_ = sb.tile([DPB, dd], BF16, name=f"xt{li}", tag="xtm", bufs=1)
                xn = sb.tile([DPB, dd], BF16, name=f"xn{li}", tag="xn", bufs=2)
                leaky(xn[:], xr[:], xt_[:])
                if li < 3:
                    nc.sync.dma_start(x_out[bs, :], xn[:])
                else:
                    nc.tensor.matmul(pool_ps[:], lhsT=gone_sb[:, b, :],
                                     rhs=xn[:], start=(b == 0),
                                     stop=(b == NB - 1))
                    if b == NB - 1:
                        psum_sb = sb.tile([G, FEAT], F32, name="psum_sb",
                                          tag="fc16", bufs=3)
                        nc.vector.tensor_copy(psum_sb[:], pool_ps[:])
                        nc.sync.dma_start(pool_in[:, :], psum_sb[:])

        gat_layer(1, EMB, D1, HEADS, RL12, x0, x1)
        gat_layer(2, D1, D2, HEADS, RL12, x1, x2)
        gat_layer(3, D2, D3, 1, RL3, x2, None)

        # =========== pooling reduce + FC ===========
        nc.gpsimd.collective_compute(
            "AllReduce", OP.add, replica_groups=[list(range(NCORES))],
            ins=[pool_in[:, :].opt()], outs=[pool_out[:, :].opt()])
        psum_all = sb.tile([G, FEAT], F32, name="psum_all", tag="fc16", bufs=3)
        nc.sync.dma_start(psum_all[:], pool_out[:, :])
        mean_f = sb.tile([G, FEAT], F32, name="mean_f", tag="fc16", bufs=3)
        nc.vector.tensor_scalar_mul(mean_f[:], psum_all[:], crecip_sb[:, 0:1])
        mean_bf = sb.tile([G, FEAT], BF16, name="mean_bf", tag="fc16", bufs=3)
        nc.vector.tensor_copy(mean_bf[:], mean_f[:])
        fc_ps = pp_dense.tile([G, 512], F32, name="fcps", tag="dps")
        for k in range(OUT // 128):
            mT_ps = pp_t.tile([128, G], BF16, name="mTps", tag="tps")
            nc.tensor.transpose(mT_ps[:], mean_bf[:, k * 128:(k + 1) * 128],
                                ident[:G, :G])
            mT = sb.tile([128, G], BF16, name="mT", tag="mT", bufs=2)
            nc.vector.tensor_copy(mT[:], mT_ps[:])
            nc.tensor.matmul(fc_ps[:], lhsT=mT[:], rhs=Wsb["Wf"][k][:],
                             start=(k == 0), stop=(k == OUT // 128 - 1))
        fc_raw = sb.tile([G, FEAT], F32, name="fc_raw", tag="fc16", bufs=3)
        nc.vector.tensor_tensor(out=fc_raw[:], in0=fc_ps[:], in1=bf_sb[:],
                                op=OP.add)
        fc_t = sb.tile([G, FEAT], F32, name="fc_t", tag="fc16", bufs=3)
        fc_o = sb.tile([G, FEAT], F32, name="fc_o", tag="fc16", bufs=3)
        leaky(fc_o[:], fc_raw[:], fc_t[:])
        nc.sync.dma_start(out_ext[:, :], fc_o[:])

    nc.finalize()
    return nc


# --------------------------------------------------------------------------
# Entry point
# --------------------------------------------------------------------------

def make_cfg(N):
    NLOC = N // NCORES
    DPB = 125 if NLOC % 125 == 0 else (64 if NLOC % 64 == 0 else NLOC)
    while NLOC % DPB != 0 or DPB > 128:
        DPB -= 1
    NB = NLOC // DPB
    NROWS = ((NLOC + 127) // 128) * 128
    return {"N": N, "NLOC": NLOC, "DPB": DPB, "NB": NB, "NROWS": NROWS}


def prepare_in_maps(inputs, cfg=None):
    x = np.asarray(inputs["x"], dtype=np.float32)
    N = x.shape[0]
    if cfg is None:
        cfg = make_cfg(N)
    cfg, metas, gone, xT, crecip = preprocess(
        x, inputs["edge_index"], inputs["batch"], cfg)

    def b16(a):
        return np.asarray(a, dtype=np.float32).astype(bf16np)

    shared = {
        "W0": np.asarray(inputs["W0"], np.float32),
        "b0": b16(inputs["b0"]).reshape(1, -1),
        "W1": b16(inputs["W1"]), "W2": b16(inputs["W2"]), "W3": b16(inputs["W3"]),
        "Wf": b16(inputs["Wf"]),
        "asd1": np.stack([b16(inputs["a1s"]).ravel(), b16(inputs["a1d"]).ravel()]),
        "asd2": np.stack([b16(inputs["a2s"]).ravel(), b16(inputs["a2d"]).ravel()]),
        "asd3": np.stack([b16(inputs["a3s"]).ravel(), b16(inputs["a3d"]).ravel()]),
        "b1": b16(inputs["b1"]).reshape(1, -1),
        "b2": b16(inputs["b2"]).reshape(1, -1),
        "b3": b16(inputs["b3"]).reshape(1, -1),
        "bfc": np.asarray(inputs["bf"], np.float32).reshape(1, -1),
        "crecip": crecip,
    }
    in_maps = []
    for c in range(NCORES):
        m = dict(shared)
        m["xT_in"] = xT[c]
        m["meta"] = metas[c]
        m["gone"] = gone[c]
        in_maps.append(m)
    return cfg, in_maps


_CACHE = {}


def _ensure_ntff_hook():
    """Register the axon NTFF profiling hook if the antenv shim is missing."""
    import sys
    import types
    try:
        from antenv.axon_hooks import get_axon_ntff_profile_hook  # noqa: F401
        return
    except ImportError:
        pass
    try:
        import antenv
        from trn_agent_boot.trn_boot import _ntff_profile_via_ctypes
    except ImportError:
        return
    mod = types.ModuleType("antenv.axon_hooks")
    mod._hook = None
    mod.set_axon_ntff_profile_hook = lambda h: setattr(mod, "_hook", h)
    mod.get_axon_ntff_profile_hook = lambda: mod._hook
    sys.modules["antenv.axon_hooks"] = mod
    antenv.axon_hooks = mod
    try:
        mod._hook = _ntff_profile_via_ctypes("/opt/axon/libaxon_pjrt.so")
    except Exception:
        mod._hook = None


def kernel(**inputs) -> np.ndarray:
    from concourse.bass_utils import run_bass_kernel_spmd
    if os.environ.get("GNN_TRACE"):
        _ensure_ntff_hook()
    cfg, in_maps = prepare_in_maps(inputs)
    key = (cfg["N"], cfg["NB"], cfg["T"])
    if key not in _CACHE:
        _CACHE[key] = build_program(cfg)
    nc = _CACHE[key]
    res = run_bass_kernel_spmd(nc, in_maps, core_ids=list(range(NCORES)),
                               trace=bool(os.environ.get("GNN_TRACE")))
    out = res.results[0]["out"]
    kernel.last_exec_time_ns = res.exec_time_ns
    kernel.last_results = res
    return np.asarray(out, dtype=np.float32)
